# revision 16
# baseline (speedup 1.0000x reference)
"""Dynamic Directional Attention on 8 trn2 NeuronCores (Bass/Tile).

Problem: B=4, L=S=2048, H=8, E=64, f32.
  qt = tanh(q * 1/(std_H(q)+eps) * dw) * dyn     (std over the HEAD dim, ddof=1)
  kt likewise; scores[b,h,l,s] = qt . kt          (contract E)
  tau[l] = sqrt(var_s(scores[l,:], ddof=1) + eps)
  A = softmax(scale * scores / tau);  out = A @ v  [B,L,H,E]

Sharding: 8 cores = 4 batches x 2 L-halves. Each core: q[b, half] = [1024, 512],
full k/v[b] = [2048, 512], all heads contiguous in the free dim. No collectives.

v3 structure (from the 377us baseline via the 373us v2):
  - transform in 2-chunk macro-tiles (halved instruction count): square+tanh
    on ACT, head-strided sum/sumsq reduces + rstd batch + apply split between
    DVE and Pool by group parity; transposes done by the DMA XBAR engine
    ([128,512] -> [128(e),4(hp),128(l)] directly into packed tkT/tqT — no PE,
    no PSUM, no drain copies).
  - Gram per pair on PE (16 accumulating matmuls via ppt banks); ksum via
    Pool tensor_reduce over tkT rows (no matmuls).
  - transposed row-stats ssqT/rsumT [l-part, (lt,h)]; m = c2/sqrt(a*(ssq -
    rsum^2/S) + eps) at [128, 64]; PE-transpose to m8b [8, L]; m broadcast
    via PE matmul (sel8 @ m8b -> mb PSUM); qts = tqT * mb.
  - phase 2 per head h, kk-interleaved on PE: st[s,l] -> PSUM [128,1024]
    (2 matmuls), Exp on ACT -> at[h] bf16; A@V of h-1: poT[65,l] +=
    Vaug^T @ at (PSUM-accumulated); poT -> bf16 SBUF cast -> DMA-XBAR
    transpose back to [l, (lt, 80)] -> batched reciprocal + broadcast-mul
    -> ob f32, one output DMA per head.
"""

import os
import sys

for _p in ("/opt/trn_rl_repo", "/root/.axon_site/_ro/trn_rl_repo"):
    if os.path.isdir(_p) and _p not in sys.path:
        sys.path.append(_p)

import numpy as np

import concourse.bass as bass
import concourse.mybir as mybir
import concourse.tile as tile
from concourse import bacc
from concourse.bass_utils import run_bass_kernel_spmd
from concourse.masks import make_identity

F32 = mybir.dt.float32
BF16 = mybir.dt.bfloat16
AF = mybir.ActivationFunctionType

B, L, S, H, E = 4, 2048, 2048, 8, 64
LC = L // 2          # 1024 l-rows per core
D = H * E            # 512 free-dim columns per core (all 8 heads)
P = 128
NLT = LC // P        # 8 l-chunks
NST = S // P         # 16 s-chunks
NHP = H // 2         # 4 head-pairs
EPS = 1e-6
SCALE = 1.0 / np.sqrt(E)
UNB_H = float(H) / float(H - 1)  # ddof=1 over heads
INV_SQRT_S = 1.0 / np.sqrt(float(S))
PO_PAD = 80          # po rows padded to a multiple of 16 for the XBAR

_last_exec_time_ns = None


def _ensure_axon_hooks():
    """Provide antenv.axon_hooks (NTFF profiling hook) if the image lacks it."""
    try:
        import antenv.axon_hooks  # noqa: F401

        return
    except ImportError:
        pass
    import contextlib
    import ctypes
    import types

    try:
        import antenv
    except ImportError:
        return

    holder = {"h": None}
    mod = types.ModuleType("antenv.axon_hooks")
    mod.set_axon_ntff_profile_hook = lambda h: holder.__setitem__("h", h)
    mod.get_axon_ntff_profile_hook = lambda: holder["h"]
    sys.modules["antenv.axon_hooks"] = mod
    antenv.axon_hooks = mod

    so_path = "/opt/axon/libaxon_pjrt.so"
    if not os.path.exists(so_path):
        return
    try:
        lib = ctypes.CDLL(so_path)
    except OSError:
        return
    if not hasattr(lib, "axon_start_nrt_profile"):
        return
    lib.axon_start_nrt_profile.argtypes = [
        ctypes.POINTER(ctypes.c_int64),
        ctypes.c_size_t,
    ]
    lib.axon_start_nrt_profile.restype = ctypes.c_int64
    lib.axon_stop_nrt_profile.argtypes = [ctypes.c_char_p]
    lib.axon_stop_nrt_profile.restype = ctypes.c_int64

    @contextlib.contextmanager
    def _hook(output_dir, device_ids):
        import jax

        jax.devices()
        if device_ids:
            ids = (ctypes.c_int64 * len(device_ids))(*device_ids)
            rc = lib.axon_start_nrt_profile(ids, len(device_ids))
        else:
            rc = lib.axon_start_nrt_profile(None, 0)
        if rc != 0:
            raise RuntimeError(f"axon_start_nrt_profile rc={rc}")
        try:
            yield
        finally:
            n = lib.axon_stop_nrt_profile(str(output_dir).encode())
            print(f"profile: {n} file(s) written to {output_dir}", file=sys.stderr)

    holder["h"] = _hook


def _chead_bcast(ap_3d, nc_, nh=H, ne=E):
    """View a [p, nc, ne] AP as [p, nc, nh, ne] with the head dim broadcast."""
    return bass.AP(
        tensor=ap_3d.tensor,
        offset=ap_3d.offset,
        ap=[list(ap_3d.ap[0]), list(ap_3d.ap[1]), [0, nh], list(ap_3d.ap[2])],
    )


def _free_bcast(ap, n):
    """Append a broadcast (step-0) dim of size n to an AP."""
    return bass.AP(
        tensor=ap.tensor,
        offset=ap.offset,
        ap=[list(d) for d in ap.ap] + [[0, n]],
    )


def build_nc():
    nc = bacc.Bacc("TRN2", target_bir_lowering=False, debug=False)
    q_d = nc.dram_tensor("q", [LC, D], F32, kind="ExternalInput")
    k_d = nc.dram_tensor("k", [S, D], F32, kind="ExternalInput")
    v_d = nc.dram_tensor("v", [S, D], F32, kind="ExternalInput")
    dw_d = nc.dram_tensor("dw", [1, 1], F32, kind="ExternalInput")
    dp_d = nc.dram_tensor("dp", [1, 1], F32, kind="ExternalInput")
    o_d = nc.dram_tensor("o", [LC, D], F32, kind="ExternalOutput")

    q_r = q_d.rearrange("(n p) d -> p n d", p=P)
    k_r = k_d.rearrange("(n p) d -> p n d", p=P)
    v_r = v_d.rearrange("(n p) d -> p n d", p=P)
    o_r = o_d.rearrange("(n p) d -> p n d", p=P)

    from contextlib import ExitStack

    with tile.TileContext(nc) as tc, ExitStack() as ctx:
        ek = ctx.enter_context
        sing = ek(tc.tile_pool(name="sing", bufs=1))
        pkn = ek(tc.tile_pool(name="kn", bufs=2))        # [128,2,512] f32 macro
        pqn = ek(tc.tile_pool(name="qn", bufs=2))
        psq = ek(tc.tile_pool(name="sq", bufs=2))        # squared macro scratch
        ptmp = ek(tc.tile_pool(name="tmp", bufs=2))      # apply-output scratch
        ptnk = ek(tc.tile_pool(name="tnk", bufs=8))      # tanh'd k macro bf16
        ptnq = ek(tc.tile_pool(name="tnq", bufs=2))
        pvn = ek(tc.tile_pool(name="vn", bufs=3))        # v nat chunks
        pvw = ek(tc.tile_pool(name="vw", bufs=2))        # wave var tiles
        pbig = ek(tc.tile_pool(name="big", bufs=1))      # tkT/tqT/qts/va8/misc
        pwsb = ek(tc.tile_pool(name="wsb", bufs=2))      # wsb / prod staging
        pat = ek(tc.tile_pool(name="at", bufs=2))        # A^T per head bf16
        pposb = ek(tc.tile_pool(name="posb", bufs=2))    # poT cast bf16 [80,1024]
        pobt = ek(tc.tile_pool(name="obt", bufs=2))      # transposed out bf16
        pob = ek(tc.tile_pool(name="ob", bufs=2))        # output staging f32
        psc = ek(tc.tile_pool(name="small", bufs=4))     # small scratch
        pps = ek(tc.tile_pool(name="ps", bufs=2, space="PSUM"))    # 2x2 banks
        ppo = ek(tc.tile_pool(name="po", bufs=1, space="PSUM"))    # 1x2 banks
        ppt = ek(tc.tile_pool(name="ptr", bufs=2, space="PSUM"))   # 2x1 bank

        # --- constants ---
        ident = sing.tile([P, P], BF16)
        make_identity(nc, ident)
        zero_t = sing.tile([P, 1], F32)
        nc.vector.memset(zero_t, 0.0)
        dw_t = sing.tile([P, 1], F32)
        nc.sync.dma_start(out=dw_t, in_=dw_d[:, :].to_broadcast([P, 1]))
        dp_t = sing.tile([P, 1], F32)
        nc.sync.dma_start(out=dp_t, in_=dp_d[:, :].to_broadcast([P, 1]))
        dp2 = sing.tile([P, 1], F32)
        nc.vector.tensor_mul(dp2, dp_t, dp_t)
        c2 = sing.tile([P, 1], F32)  # scale * dyn^2
        nc.vector.tensor_scalar_mul(c2, dp2, float(SCALE))
        a_t = sing.tile([P, 1], F32)  # dyn^4 / (S-1)   (tau^2 coeff)
        nc.vector.tensor_mul(a_t, dp2, dp2)
        nc.vector.tensor_scalar_mul(a_t, a_t, 1.0 / (S - 1.0))
        eps_t = sing.tile([P, 1], F32)
        nc.vector.memset(eps_t, EPS)
        ones2 = sing.tile([P, 2], BF16)  # block ones for per-head column sums
        nc.vector.memset(ones2, 0.0)
        nc.vector.memset(ones2[0:E, 0:1], 1.0)
        nc.vector.memset(ones2[E:P, 1:2], 1.0)
        # mb broadcast selectors: sel8[x, hp*128 + j*64 + e] = 1 iff 2hp+j == x
        sel8 = sing.tile([8, NHP, P], BF16)
        sel8f = sel8.rearrange("p a b -> p (a b)")
        nc.gpsimd.memset(sel8f, 1.0)
        nc.gpsimd.affine_select(out=sel8f, in_=sel8f,
                                compare_op=mybir.AluOpType.is_ge, fill=0.0,
                                base=0, pattern=[[1, NHP * P]],
                                channel_multiplier=-E)
        nc.gpsimd.affine_select(out=sel8f, in_=sel8f,
                                compare_op=mybir.AluOpType.is_ge, fill=0.0,
                                base=E - 1, pattern=[[-1, NHP * P]],
                                channel_multiplier=E)

        # --- big persistent tiles ---
        tkT = pbig.tile([P, NHP, S], BF16, tag="tkT")         # 16 KB/part
        tqT = pbig.tile([P, NHP, LC], BF16, tag="tqT")        # 8 KB/part
        qts = pbig.tile([P, NHP, LC], BF16, tag="qts")        # 8 KB/part
        va8 = pbig.tile([P, NST, H, E + 1], BF16, tag="va8")  # 16.25 KB/part
        gsb = pbig.tile([P, NHP, P], BF16, tag="gsb")         # 1 KB/part
        k2 = pbig.tile([P, NHP, 2], BF16, tag="k2")
        nc.vector.memset(k2, 0.0)
        m8b = pbig.tile([8, LC], BF16, tag="m8b")

        # ---------------------------------------------------------------
        # transform waves: 2-chunk macro-tiles (NM macro groups of 2)
        # ---------------------------------------------------------------
        def transform_wave(src_r, i0, nmac, is_q, wave_idx):
            """Process chunks [i0, i0+2*nmac) as nmac 2-chunk macro tiles."""
            out_list = []
            vv_s = pvw.tile([P, nmac, 2, E], F32, tag="vs")
            vv_q = pvw.tile([P, nmac, 2, E], F32, tag="vq")
            # half-wave barrier granularity = 2 macro groups (4 chunks) to
            # match the 2-deep macro rings.
            HWM = 2
            mac_stats = []
            for g0 in range(0, nmac, HWM):
                for g in range(g0, min(g0 + HWM, nmac)):
                    i = i0 + 2 * g
                    if is_q:
                        nat = pqn.tile([P, 2, D], F32, tag="qn")
                    else:
                        nat = pkn.tile([P, 2, D], F32, tag="kn")
                    nc.sync.dma_start(out=nat, in_=src_r[:, i : i + 2, :])
                    sq = psq.tile([P, 2, D], F32, tag="sq")
                    nc.scalar.activation(
                        sq.rearrange("p a b -> p (a b)"),
                        nat.rearrange("p a b -> p (a b)"),
                        AF.Square, bias=zero_t, scale=1.0)
                    # head-strided sums (free-axis reduce is DVE-only)
                    red0 = red1 = nc.vector
                    red0.tensor_reduce(
                        vv_s[:, g, :, :],
                        nat.rearrange("p c (h e) -> p c e h", h=H),
                        axis=mybir.AxisListType.X, op=mybir.AluOpType.add)
                    red1.tensor_reduce(
                        vv_q[:, g, :, :],
                        sq.rearrange("p c (h e) -> p c e h", h=H),
                        axis=mybir.AxisListType.X, op=mybir.AluOpType.add)
                    mac_stats.append((g, i, nat))
                # rstd batch for this half-wave:
                # vv_q <- 1/(sqrt((vv_q - vv_s^2/H)*UNB_H/H)+eps)
                g1 = min(g0 + HWM, nmac)
                flat_s = vv_s[:, g0:g1, :, :].rearrange("p a c e -> p (a c e)")
                flat_q = vv_q[:, g0:g1, :, :].rearrange("p a c e -> p (a c e)")
                nc.vector.tensor_mul(flat_s, flat_s, flat_s)
                nc.vector.tensor_scalar_mul(flat_s, flat_s, 1.0 / H)
                nc.vector.tensor_sub(flat_q, flat_q, flat_s)
                nc.scalar.activation(flat_q, flat_q, AF.Sqrt,
                                     bias=zero_t, scale=UNB_H / H)
                nc.vector.tensor_scalar_add(flat_q, flat_q, EPS)
                nc.vector.reciprocal(flat_q, flat_q)
            # apply + tanh + DMA-XBAR transpose per macro group
            dstT = tqT if is_q else tkT
            for g, i, nat in mac_stats:
                tmp = ptmp.tile([P, 2, D], F32, tag="tmp")
                app = nc.gpsimd if g % 2 == 0 else nc.vector
                app.tensor_mul(tmp, nat, _chead_bcast(vv_q[:, g, :, :], 2))
                if is_q:
                    tn = ptnq.tile([P, 2, D], BF16, tag="tnq")
                else:
                    tn = ptnk.tile([P, 2, D], BF16, tag="tnk")
                nc.scalar.activation(
                    tn.rearrange("p a b -> p (a b)"),
                    tmp.rearrange("p a b -> p (a b)"),
                    AF.Tanh, bias=zero_t, scale=dw_t)
                # XBAR transpose per chunk: [128, 512] -> [128(e), 4(hp), 128(l)]
                for c in range(2):
                    nc.sync.dma_start(
                        out=dstT[:, :, (i + c) * P : (i + c + 1) * P],
                        in_=tn[:, c, :], transpose=True)
                out_list.append((i, tn))
            return out_list

        tk_chunks = []
        tk_chunks += transform_wave(k_r, 0, 4, False, 0)
        tk_chunks += transform_wave(k_r, 8, 4, False, 1)

        # --- Gram per pair (PE; overlaps q-wave on other engines) ---
        for hp in range(NHP):
            g_a = ppt.tile([P, P], F32, tag="tr")
            idx = 0
            for i, tn in tk_chunks:
                for c in range(2):
                    lhs = tn[:, c, hp * P : (hp + 1) * P]
                    nc.tensor.matmul(g_a, lhs, lhs,
                                     start=(idx == 0), stop=(idx == NST - 1))
                    idx += 1
            if hp % 2 == 0:
                nc.vector.tensor_copy(gsb[:, hp, :], g_a)
            else:
                nc.scalar.copy(gsb[:, hp, :], g_a)
            nc.vector.memset(gsb[0:E, hp, E:P], 0.0)
            nc.vector.memset(gsb[E:P, hp, 0:E], 0.0)

        # ksum via DVE row-reduce over tkT (k2 = ks/sqrt(S), block layout)
        ks4 = pbig.tile([P, NHP], F32, tag="ks4")
        for hp in range(NHP):
            nc.vector.tensor_reduce(ks4[:, hp : hp + 1], tkT[:, hp, :],
                                    axis=mybir.AxisListType.X,
                                    op=mybir.AluOpType.add)
        for hp in range(NHP):
            nc.vector.tensor_scalar_mul(k2[0:E, hp, 0:1], ks4[0:E, hp : hp + 1],
                                        INV_SQRT_S)
            nc.vector.tensor_scalar_mul(k2[E:P, hp, 1:2], ks4[E:P, hp : hp + 1],
                                        INV_SQRT_S)

        tq_chunks = transform_wave(q_r, 0, 4, True, 0)

        # --- transposed row-stats: ssqT/rsumT [l-part, (lt, h)] ---
        ssqT = ppt.tile([P, NLT, H], F32, tag="tr")
        rsumT = ppt.tile([P, NLT, H], F32, tag="tr")
        for hp in range(NHP):
            wps = pps.tile([P, LC], F32, tag="st")
            for jh in range(2):
                nc.tensor.matmul(wps[:, jh * 512 : (jh + 1) * 512],
                                 gsb[:, hp, :],
                                 tqT[:, hp, jh * 512 : (jh + 1) * 512],
                                 start=True, stop=True)
            wsb = pwsb.tile([P, LC], BF16, tag="wsb")
            nc.scalar.copy(wsb, wps)
            prod = pwsb.tile([P, LC], BF16, tag="prod")
            nc.vector.tensor_mul(prod, tqT[:, hp, :], wsb)
            for lt in range(NLT):
                nc.tensor.matmul(ssqT[:, lt, 2 * hp : 2 * hp + 2],
                                 prod[:, lt * P : (lt + 1) * P], ones2,
                                 start=True, stop=True)
                nc.tensor.matmul(rsumT[:, lt, 2 * hp : 2 * hp + 2],
                                 tqT[:, hp, lt * P : (lt + 1) * P],
                                 k2[:, hp, :],
                                 start=True, stop=True)

        # m = c2 / sqrt(a*(ssq - rsum^2/S) + eps), computed at [128, 64]
        mT = psc.tile([P, NLT * H], F32, tag="mT")
        ssqf = ssqT.rearrange("p a b -> p (a b)")
        rsumf = rsumT.rearrange("p a b -> p (a b)")
        nc.scalar.activation(mT, rsumf, AF.Square, bias=zero_t, scale=1.0)
        nc.vector.tensor_sub(mT, ssqf, mT)
        nc.scalar.activation(mT, mT, AF.Sqrt, bias=eps_t, scale=a_t)
        nc.vector.reciprocal(mT, mT)
        mTb = psc.tile([P, NLT, H], BF16, tag="mTb")
        nc.vector.tensor_scalar_mul(mTb.rearrange("p a b -> p (a b)"), mT, c2)
        # transpose to m8b [8, (lt, p)] = [8, LC]: 8 per-lt PE transposes
        mtr = ppt.tile([8, NLT, P], BF16, tag="tr")
        for lt in range(NLT):
            nc.tensor.transpose(mtr[:, lt, :], mTb[:, lt, :], ident)
        nc.vector.tensor_copy(m8b, mtr.rearrange("p a b -> p (a b)"))

        # --- phase 2 ---
        nc.vector.memset(va8[:, :, :, E : E + 1], 1.0)
        for kk in range(NST):
            vn = pvn.tile([P, D], F32, tag="vn")
            nc.sync.dma_start(out=vn, in_=v_r[:, kk, :])
            nc.gpsimd.tensor_copy(va8[:, kk, :, 0:E],
                                  vn.rearrange("p (h e) -> p h e", h=H))

        def emit_qts(hp):
            mb = pps.tile([P, LC], F32, tag="st")
            for jh in range(2):
                nc.tensor.matmul(mb[:, jh * 512 : (jh + 1) * 512],
                                 sel8[:, hp, :],
                                 m8b[:, jh * 512 : (jh + 1) * 512],
                                 start=True, stop=True)
            nc.vector.tensor_mul(qts[:, hp, :], tqT[:, hp, :], mb)

        at_by_head = {}
        po_by_head = {}

        def emit_av_group(h, at, po, kk):
            for jh in range(2):
                nc.tensor.matmul(po[:, jh * 512 : (jh + 1) * 512],
                                 va8[:, kk, h, :],
                                 at[:, kk, jh * 512 : (jh + 1) * 512],
                                 start=(kk == 0), stop=(kk == NST - 1))

        def emit_av_drain(h, po):
            # cast poT [65, LC] f32 PSUM -> [80, LC] bf16 SBUF (rows 65:80
            # zero-filled so the XBAR reads defined data)
            po_sb = pposb.tile([PO_PAD, LC], BF16, tag="posb")
            nc.vector.memset(po_sb[E : PO_PAD, :], 0.0)
            nc.vector.tensor_copy(po_sb[0 : E + 1, :], po)
            po_by_head[h] = po_sb

        def emit_out(h):
            po_sb = po_by_head.pop(h)
            obt = pobt.tile([P, NLT, PO_PAD], BF16, tag="obt")
            nc.sync.dma_start(out=obt, in_=po_sb, transpose=True)
            rec = psc.tile([P, NLT], F32, tag="rec")
            nc.vector.reciprocal(rec, obt[:, :, E])
            ob = pob.tile([P, NLT, E], F32, tag="ob")
            nc.vector.tensor_mul(ob, obt[:, :, 0:E], _free_bcast(rec, E))
            nc.sync.dma_start(out=o_r[:, :, h * E : (h + 1) * E], in_=ob)

        emit_qts(0)
        for h in range(H):
            if h % 2 == 0 and h > 0:
                emit_qts(h // 2)
            at = pat.tile([P, NST, LC], BF16, tag="at")
            at_by_head[h] = at
            hp, local = h // 2, h % 2
            off = local * E
            atp = at_by_head.get(h - 1)
            if atp is not None:
                po = ppo.tile([E + 1, LC], F32, tag="po")
            for kk in range(NST):
                st_ps = pps.tile([P, LC], F32, tag="st")
                for jh in range(2):
                    nc.tensor.matmul(
                        st_ps[:, jh * 512 : (jh + 1) * 512],
                        tkT[off : off + E, hp, kk * P : (kk + 1) * P],
                        qts[off : off + E, hp, jh * 512 : (jh + 1) * 512],
                        start=True, stop=True,
                    )
                if atp is not None:
                    emit_av_group(h - 1, atp, po, kk)
                nc.scalar.activation(at[:, kk, :], st_ps, AF.Exp,
                                     bias=zero_t, scale=1.0)
            if atp is not None:
                emit_av_drain(h - 1, po)
            if h >= 2:
                emit_out(h - 2)
        # tail: av(7) + out(6) + out(7)
        at7 = at_by_head[H - 1]
        po = ppo.tile([E + 1, LC], F32, tag="po")
        for kk in range(NST):
            emit_av_group(H - 1, at7, po, kk)
        emit_av_drain(H - 1, po)
        emit_out(H - 2)
        emit_out(H - 1)

    return nc


_nc_cache = None


def kernel(queries, keys, values, attn_mask=None, directional_weights=None,
           dynamic_param=None, **_unused):
    global _nc_cache, _last_exec_time_ns
    q = np.asarray(queries, dtype=np.float32)
    k = np.asarray(keys, dtype=np.float32)
    v = np.asarray(values, dtype=np.float32)
    dw = np.asarray(directional_weights, dtype=np.float32).reshape(1, 1)
    dp = np.asarray(dynamic_param, dtype=np.float32).reshape(1, 1)

    if _nc_cache is None:
        nc = build_nc()
        nc.finalize()
        _nc_cache = nc
    nc = _nc_cache

    in_maps = []
    for c in range(8):
        b, lh = c // 2, c % 2
        in_maps.append({
            "q": np.ascontiguousarray(q[b, lh * LC : (lh + 1) * LC]).reshape(LC, D),
            "k": np.ascontiguousarray(k[b]).reshape(S, D),
            "v": np.ascontiguousarray(v[b]).reshape(S, D),
            "dw": dw, "dp": dp,
        })

    tracing = bool(os.environ.get("BASS_TRACE"))
    if tracing:
        _ensure_axon_hooks()
        import concourse.bass_utils as _bu

        _orig_upload = _bu.upload_artifacts
        _bu.upload_artifacts = lambda d: d  # no bucket access in this sandbox
        try:
            res = run_bass_kernel_spmd(nc, in_maps, core_ids=list(range(8)))
        except Exception as e:  # fall back to an untraced run
            print(f"traced run failed ({e!r}); retrying untraced", file=sys.stderr)
            os.environ["BASS_NEVER_TRACE"] = "1"
            try:
                res = run_bass_kernel_spmd(nc, in_maps, core_ids=list(range(8)))
            finally:
                os.environ.pop("BASS_NEVER_TRACE", None)
        finally:
            _bu.upload_artifacts = _orig_upload
    else:
        res = run_bass_kernel_spmd(nc, in_maps, core_ids=list(range(8)))
    _last_exec_time_ns = res.exec_time_ns

    out = np.empty((B, L, H, E), dtype=np.float32)
    for c in range(8):
        b, lh = c // 2, c % 2
        out[b, lh * LC : (lh + 1) * LC] = res.results[c]["o"].reshape(LC, H, E)
    return out
